# revision 7
# baseline (speedup 1.0000x reference)
r"""Trainium2 Bass kernel for the triangular-DP "MAA layer" problem.

Reference computes, per frame t (T=1024, D=256, L=T+1 counts):
    q_t = (1-p_t) q_{t-1} + p_t shift(q_{t-1})          (Poisson-binomial DP)
    m_t = p_t a m_sh + (1-p_t) m + p_t b q_sh x_t       ([L, D] state)
    out = sum_i m_T[i, :]                               ([D])

Algebraic restructuring used here: with s_t[i] = i*m_t[i], every step is a
polynomial in the (commuting, nilpotent) count-shift operator S, so the whole
scan collapses to

    out[d] = sum_t c_t x[t, d],
    c_t    = p_t * I_t,   I_t = \int_0^1 prod_{s != t} ((1-p_s) + p_s u) du.

The integrand is a degree-(T-1) polynomial, so K-node Gauss-Legendre quadrature
with K >= T/2 is exact; K=128 is already converged to the f32 noise floor
(verified < 3e-6 rel err vs the reference). With f[t,k] = (1-p_t) + p_t u_k:

    G_k = prod_t f[t,k]  (computed as exp(sum_t ln f[t,k]))
    c_t = p_t * sum_k (w_k G_k) / f[t,k]
    out = c^T @ x

This is ~30 small instructions per core: no sequential scan remains. The 8
cores each run the identical full problem (1 MB x DMA each); replicating is
faster than any sharded variant because the 8-core collective latency floor
(~5-10 us) exceeds the whole kernel's runtime.
"""

import numpy as np

T, D, NCH, P, K = 1024, 256, 8, 128, 128
N_CORES = 8

_CACHE = {}


def _gl_nodes_weights():
    nodes, weights = np.polynomial.legendre.leggauss(K)
    u = (nodes + 1.0) * 0.5
    w = weights * 0.5
    return u, w


def _build_program():
    import concourse.bass as bass
    import concourse.bacc as bacc
    import concourse.mybir as mybir
    import concourse.tile as tile

    f32 = mybir.dt.float32
    A = mybir.AluOpType
    ACT = mybir.ActivationFunctionType

    nc = bacc.Bacc("TRN2", target_bir_lowering=False, debug=False,
                   num_devices=N_CORES)

    # aux columns: [um1 (K) | pcol (NCH) | onescol (1)] → one DMA, one sem wait
    AUXW = K + NCH + 1
    xall_d = nc.dram_tensor("xall", [P, NCH * D], f32, kind="ExternalInput")
    aux_d = nc.dram_tensor("aux", [P, AUXW], f32, kind="ExternalInput")
    rows_d = nc.dram_tensor("rows", [1, 2 * K], f32, kind="ExternalInput")
    out_d = nc.dram_tensor("out", [1, D], f32, kind="ExternalOutput")

    with tile.TileContext(nc) as tc:
        with (
            tc.tile_pool(name="sb", bufs=1) as sb,
            tc.tile_pool(name="ps", bufs=1, space=bass.MemorySpace.PSUM) as ps,
        ):
            xall = sb.tile([P, NCH * D], f32, tag="xall")
            aux = sb.tile([P, AUXW], f32, tag="aux")
            rows = sb.tile([1, 2 * K], f32, tag="rows")
            nc.sync.dma_start(aux[:], aux_d[:])
            nc.sync.dma_start(rows[:], rows_d[:])
            nc.sync.dma_start(xall[:], xall_d[:])
            um1 = aux[:, 0:K]
            pcol = aux[:, K:K + NCH]
            onescol = aux[:, K + NCH:K + NCH + 1]

            # Stage A: f_c = 1 + p_c * (u - 1);  slog = sum_t ln f  (PE-accumulated)
            slog_ps = ps.tile([1, K], f32, tag="slog")
            fs = []
            for c in range(NCH):
                f_c = sb.tile([P, K], f32, tag=f"f{c}")
                nc.vector.tensor_scalar(f_c[:], um1[:], pcol[:, c:c + 1], 1.0,
                                        op0=A.mult, op1=A.add)
                lf = sb.tile([P, K], f32, tag="lf", bufs=2)
                nc.scalar.activation(lf[:], f_c[:], ACT.Ln)
                nc.tensor.matmul(slog_ps[:], onescol[:], lf[:],
                                 start=(c == 0), stop=(c == NCH - 1))
                fs.append(f_c)

            # G = exp(slog); gw = w * G; broadcast to all partitions via PE
            g = sb.tile([1, K], f32, tag="g")
            nc.scalar.activation(g[:], slog_ps[:], ACT.Exp)
            gw = sb.tile([1, K], f32, tag="gw")
            nc.vector.tensor_tensor(gw[:], g[:], rows[:, 0:K], op=A.mult)
            gwbc_ps = ps.tile([P, K], f32, tag="gwbc")
            nc.tensor.matmul(gwbc_ps[:], rows[:, K:2 * K], gw[:],
                             start=True, stop=True)
            gwbc = sb.tile([P, K], f32, tag="gwbcsb")
            nc.vector.tensor_copy(gwbc[:], gwbc_ps[:])

            # Stage B: c_c = p_c * sum_k gw_k / f_c ; out += c_c^T @ x_c
            out_ps = ps.tile([1, D], f32, tag="out")
            scr = sb.tile([P, K], f32, tag="scr")
            for c in range(NCH):
                rc = sb.tile([P, K], f32, tag="recip", bufs=2)
                nc.vector.reciprocal(rc[:], fs[c][:])
                cfin = sb.tile([P, 1], f32, tag="cfin", bufs=2)
                nc.vector.scalar_tensor_tensor(scr[:], rc[:], pcol[:, c:c + 1],
                                               gwbc[:], op0=A.mult, op1=A.mult,
                                               accum_out=cfin[:])
                nc.tensor.matmul(out_ps[:], cfin[:], xall[:, c * D:(c + 1) * D],
                                 start=(c == 0), stop=(c == NCH - 1))

            out_sb = sb.tile([1, D], f32, tag="outsb")
            nc.vector.tensor_copy(out_sb[:], out_ps[:])
            nc.sync.dma_start(out_d[:], out_sb[:])

    nc.compile()
    return nc


def _make_in_map(p, x):
    p = np.ascontiguousarray(np.asarray(p, dtype=np.float32)).reshape(T)
    x = np.ascontiguousarray(np.asarray(x, dtype=np.float32)).reshape(T, D)
    u, w = _gl_nodes_weights()
    um1bc = np.tile((u - 1.0).astype(np.float32)[None, :], (P, 1))
    pcol = np.ascontiguousarray(p.reshape(NCH, P).T)
    onescol = np.ones((P, 1), np.float32)
    aux = np.ascontiguousarray(
        np.concatenate([um1bc, pcol, onescol], axis=1))
    rows = np.concatenate([w.astype(np.float32),
                           np.ones(K, np.float32)]).reshape(1, 2 * K)
    xall = np.ascontiguousarray(
        x.reshape(NCH, P, D).transpose(1, 0, 2).reshape(P, NCH * D))
    return {
        "xall": xall,
        "aux": aux,
        "rows": rows,
    }


def _run(p, x, trace=False, tmpdir=None):
    from concourse.bass_utils import run_bass_kernel_spmd

    if "nc" not in _CACHE:
        _CACHE["nc"] = _build_program()
    nc = _CACHE["nc"]
    in_map = _make_in_map(p, x)
    in_maps = [in_map for _ in range(N_CORES)]
    res = run_bass_kernel_spmd(nc, in_maps, list(range(N_CORES)),
                               trace=trace, tmpdir=tmpdir)
    out = np.asarray(res.results[0]["out"], dtype=np.float32).reshape(D)
    return out, res


def kernel(p, x):
    out, _ = _run(p, x, trace=False)
    return out


# revision 10
# speedup vs baseline: 1.1260x; 1.1260x over previous
r"""Trainium2 Bass kernel for the triangular-DP "MAA layer" problem.

Reference computes, per frame t (T=1024, D=256, L=T+1 counts):
    q_t = (1-p_t) q_{t-1} + p_t shift(q_{t-1})          (Poisson-binomial DP)
    m_t = p_t a m_sh + (1-p_t) m + p_t b q_sh x_t       ([L, D] state)
    out = sum_i m_T[i, :]                               ([D])

Algebraic restructuring used here: with s_t[i] = i*m_t[i], every step is a
polynomial in the (commuting, nilpotent) count-shift operator S, so the whole
scan collapses to

    out[d] = sum_t c_t x[t, d],
    c_t    = p_t * I_t,   I_t = int_0^1 prod_{s != t} ((1-p_s) + p_s u) du.

The integrand is a degree-(T-1) polynomial; K-node Gauss-Legendre quadrature
is exact for K >= T/2 and already converged to the f32 noise floor at K=64
(verified < 3e-6 rel err vs the reference). With f[t,k] = (1-p_t) + p_t u_k:

    G_k = prod_t f[t,k]  (as exp(sum_t ln f))
    c_t = p_t * sum_k (w_k G_k) / f[t,k]
    out = c^T @ x

Device mapping (t on partitions, 8 chunks of 128; k on free dim, K=64):
  - lf[c] = Ln(um1 * p_c + 1)      one fused ScalarE activation per chunk
  - slog  = ones^T @ lf_supertile  one PE matmul; cross-chunk sum on DVE
  - G     = Exp(slog), gw = w*G, broadcast via 1-partition PE matmul
  - rf[c] = Exp(-lf[c])            reciprocal on ScalarE (table reuse)
  - cfin[c] = sum_k (rf*p_c)*gwbc  one DVE scalar_tensor_tensor w/ accum
  - z     = sum_c cfin_c * x_c     DVE chain; out = ones^T @ z (one matmul)

The 8 cores each run the identical full problem (1 MB x DMA each);
replication beats sharding because the 8-core collective latency floor
(~5-10 us) exceeds the whole compute phase.
"""

import numpy as np

T, D, NCH, P, K = 1024, 256, 8, 128, 64
N_CORES = 8

_CACHE = {}


def _gl_nodes_weights():
    nodes, weights = np.polynomial.legendre.leggauss(K)
    u = (nodes + 1.0) * 0.5
    w = weights * 0.5
    return u, w


def _build_program():
    import concourse.bass as bass
    import concourse.bacc as bacc
    import concourse.mybir as mybir
    import concourse.tile as tile

    f32 = mybir.dt.float32
    A = mybir.AluOpType
    ACT = mybir.ActivationFunctionType

    nc = bacc.Bacc("TRN2", target_bir_lowering=False, debug=False,
                   num_devices=N_CORES)

    # aux columns: [um1 (K) | pcol (NCH) | onescol (1)] -> one DMA, one wait
    AUXW = K + NCH + 1
    xall_d = nc.dram_tensor("xall", [P, NCH * D], f32, kind="ExternalInput")
    aux_d = nc.dram_tensor("aux", [P, AUXW], f32, kind="ExternalInput")
    rows_d = nc.dram_tensor("rows", [1, K + P], f32, kind="ExternalInput")
    out_d = nc.dram_tensor("out", [1, D], f32, kind="ExternalOutput")

    with tile.TileContext(nc) as tc:
        with (
            tc.tile_pool(name="sb", bufs=1) as sb,
            tc.tile_pool(name="ps", bufs=1, space=bass.MemorySpace.PSUM) as ps,
        ):
            xall = sb.tile([P, NCH * D], f32, tag="xall")
            aux = sb.tile([P, AUXW], f32, tag="aux")
            rows = sb.tile([1, K + P], f32, tag="rows")
            nc.sync.dma_start(aux[:], aux_d[:])
            nc.sync.dma_start(rows[:], rows_d[:])
            nc.sync.dma_start(xall[:], xall_d[:])
            um1 = aux[:, 0:K]
            pcol = aux[:, K:K + NCH]
            onescol = aux[:, K + NCH:K + NCH + 1]

            # Stage A: lf_c = Ln(p_c * um1 + 1) fused on ScalarE
            lfbig = sb.tile([P, NCH * K], f32, tag="lfbig")
            for c in range(NCH):
                nc.scalar.activation(lfbig[:, c * K:(c + 1) * K], um1[:],
                                     ACT.Ln, bias=1.0,
                                     scale=pcol[:, c:c + 1])

            # slog over t: one 512-col matmul, then cross-chunk sum on DVE
            slogrow_ps = ps.tile([1, NCH * K], f32, tag="slogrow")
            nc.tensor.matmul(slogrow_ps[:], onescol[:], lfbig[:],
                             start=True, stop=True)
            slog = sb.tile([1, K], f32, tag="slog")
            nc.vector.tensor_reduce(
                slog[:],
                slogrow_ps.rearrange("a (c k) -> a k c", c=NCH),
                axis=mybir.AxisListType.X, op=A.add)

            # G = exp(slog); gw = w * G; broadcast to all partitions via PE
            g = sb.tile([1, K], f32, tag="g")
            nc.scalar.activation(g[:], slog[:], ACT.Exp)
            gw = sb.tile([1, K], f32, tag="gw")
            nc.vector.tensor_tensor(gw[:], g[:], rows[:, 0:K], op=A.mult)
            gwbc_ps = ps.tile([P, K], f32, tag="gwbc")
            nc.tensor.matmul(gwbc_ps[:], rows[:, K:K + P], gw[:],
                             start=True, stop=True)

            # Stage B: rf = exp(-lf) = 1/f on ScalarE;
            # cfin_c = sum_k (rf * p_c) * gwbc;  z += cfin_c * x_c on DVE
            scr = sb.tile([P, K], f32, tag="scr")
            zpp = [sb.tile([P, D], f32, tag="z0", name="z0"),
                   sb.tile([P, D], f32, tag="z1", name="z1")]
            for c in range(NCH):
                rf = sb.tile([P, K], f32, tag=f"rf{c % 2}")
                nc.scalar.activation(rf[:], lfbig[:, c * K:(c + 1) * K],
                                     ACT.Exp, scale=-1.0)
                cfin = sb.tile([P, 1], f32, tag=f"cf{c % 2}")
                nc.vector.scalar_tensor_tensor(scr[:], rf[:],
                                               pcol[:, c:c + 1], gwbc_ps[:],
                                               op0=A.mult, op1=A.mult,
                                               accum_out=cfin[:])
                xc = xall[:, c * D:(c + 1) * D]
                if c == 0:
                    nc.vector.tensor_scalar_mul(zpp[0][:], xc, cfin[:])
                else:
                    nc.vector.scalar_tensor_tensor(zpp[c % 2][:], xc, cfin[:],
                                                   zpp[(c - 1) % 2][:],
                                                   op0=A.mult, op1=A.add)

            # out = ones^T @ z  (single 256-col matmul), then DMA out
            out_ps = ps.tile([1, D], f32, tag="out")
            nc.tensor.matmul(out_ps[:], onescol[:], zpp[(NCH - 1) % 2][:],
                             start=True, stop=True)
            out_sb = sb.tile([1, D], f32, tag="outsb")
            nc.vector.tensor_copy(out_sb[:], out_ps[:])
            nc.sync.dma_start(out_d[:], out_sb[:])

    nc.compile()
    return nc


def _make_in_map(p, x):
    p = np.ascontiguousarray(np.asarray(p, dtype=np.float32)).reshape(T)
    x = np.ascontiguousarray(np.asarray(x, dtype=np.float32)).reshape(T, D)
    u, w = _gl_nodes_weights()
    um1bc = np.tile((u - 1.0).astype(np.float32)[None, :], (P, 1))
    pcol = np.ascontiguousarray(p.reshape(NCH, P).T)
    onescol = np.ones((P, 1), np.float32)
    aux = np.ascontiguousarray(
        np.concatenate([um1bc, pcol, onescol], axis=1))
    rows = np.concatenate([w.astype(np.float32),
                           np.ones(P, np.float32)]).reshape(1, K + P)
    xall = np.ascontiguousarray(
        x.reshape(NCH, P, D).transpose(1, 0, 2).reshape(P, NCH * D))
    return {
        "xall": xall,
        "aux": aux,
        "rows": rows,
    }


def _run(p, x, trace=False, tmpdir=None):
    from concourse.bass_utils import run_bass_kernel_spmd

    if "nc" not in _CACHE:
        _CACHE["nc"] = _build_program()
    nc = _CACHE["nc"]
    in_map = _make_in_map(p, x)
    in_maps = [in_map for _ in range(N_CORES)]
    res = run_bass_kernel_spmd(nc, in_maps, list(range(N_CORES)),
                               trace=trace, tmpdir=tmpdir)
    out = np.asarray(res.results[0]["out"], dtype=np.float32).reshape(D)
    return out, res


def kernel(p, x):
    out, _ = _run(p, x, trace=False)
    return out


# revision 12
# speedup vs baseline: 1.1802x; 1.0481x over previous
r"""Trainium2 Bass kernel for the triangular-DP "MAA layer" problem.

Reference computes, per frame t (T=1024, D=256, L=T+1 counts):
    q_t = (1-p_t) q_{t-1} + p_t shift(q_{t-1})          (Poisson-binomial DP)
    m_t = p_t a m_sh + (1-p_t) m + p_t b q_sh x_t       ([L, D] state)
    out = sum_i m_T[i, :]                               ([D])

Algebraic restructuring used here: with s_t[i] = i*m_t[i], every step is a
polynomial in the (commuting, nilpotent) count-shift operator S, so the whole
scan collapses to

    out[d] = sum_t c_t x[t, d],
    c_t    = p_t * I_t,   I_t = int_0^1 prod_{s != t} ((1-p_s) + p_s u) du.

The integrand is a degree-(T-1) polynomial; K-node Gauss-Legendre quadrature
is exact for K >= T/2 and already converged to the f32 noise floor at K=64
(verified < 3e-6 rel err vs the reference). With f[t,k] = (1-p_t) + p_t u_k:

    G_k = prod_t f[t,k]  (as exp(sum_t ln f))
    c_t = p_t * sum_k (w_k G_k) / f[t,k]
    out = c^T @ x

Device mapping (t on partitions, 8 chunks of 128; k on free dim, K=64):
  - lf[c] = Ln(um1 * p_c + 1)      one fused ScalarE activation per chunk
  - slog  = ones^T @ lf_supertile  one PE matmul; cross-chunk sum on DVE
  - G     = Exp(slog), gw = w*G, broadcast via 1-partition PE matmul
  - rf[c] = Exp(-lf[c])            reciprocal on ScalarE (table reuse)
  - cfin[c] = sum_k (rf*p_c)*gwbc  one DVE scalar_tensor_tensor w/ accum
  - z     = sum_c cfin_c * x_c     DVE chain; out = ones^T @ z (one matmul)

The 8 cores each run the identical full problem (1 MB x DMA each);
replication beats sharding because the 8-core collective latency floor
(~5-10 us) exceeds the whole compute phase.
"""

import numpy as np

T, D, NCH, P, K = 1024, 256, 8, 128, 64
N_CORES = 8

_CACHE = {}


def _gl_nodes_weights():
    nodes, weights = np.polynomial.legendre.leggauss(K)
    u = (nodes + 1.0) * 0.5
    w = weights * 0.5
    return u, w


def _build_program():
    import concourse.bass as bass
    import concourse.bacc as bacc
    import concourse.mybir as mybir
    import concourse.tile as tile

    f32 = mybir.dt.float32
    A = mybir.AluOpType
    ACT = mybir.ActivationFunctionType

    nc = bacc.Bacc("TRN2", target_bir_lowering=False, debug=False,
                   num_devices=N_CORES)

    # aux columns: [um1 (K) | pcol (NCH) | onescol (1)] -> one DMA, one wait
    AUXW = K + NCH + 1
    xall_d = nc.dram_tensor("xall", [P, NCH * D], f32, kind="ExternalInput")
    aux_d = nc.dram_tensor("aux", [P, AUXW], f32, kind="ExternalInput")
    rows_d = nc.dram_tensor("rows", [1, K + P], f32, kind="ExternalInput")
    out_d = nc.dram_tensor("out", [1, D], f32, kind="ExternalOutput")

    with tile.TileContext(nc) as tc:
        with (
            tc.tile_pool(name="sb", bufs=1) as sb,
            tc.tile_pool(name="ps", bufs=1, space=bass.MemorySpace.PSUM) as ps,
        ):
            xall = sb.tile([P, NCH * D], f32, tag="xall")
            aux = sb.tile([P, AUXW], f32, tag="aux")
            rows = sb.tile([1, K + P], f32, tag="rows")
            nc.sync.dma_start(aux[:], aux_d[:])
            nc.sync.dma_start(rows[:], rows_d[:])
            nc.sync.dma_start(xall[:], xall_d[:])
            um1 = aux[:, 0:K]
            pcol = aux[:, K:K + NCH]
            onescol = aux[:, K + NCH:K + NCH + 1]

            # Stage A, all chunks batched via stride-0 broadcast APs:
            #   fm1[t,(c,k)] = p[c,t] * um1[k]  (one DVE op)
            #   lf = Ln(fm1 + 1)                (one ScalarE op, bias fused)
            um1_rep = um1.unsqueeze(1).broadcast_to([P, NCH, K])
            p_rep = pcol.unsqueeze(2).broadcast_to([P, NCH, K])
            fm1 = sb.tile([P, NCH * K], f32, tag="fm1")
            nc.vector.tensor_tensor(fm1.rearrange("p (c k) -> p c k", c=NCH),
                                    um1_rep, p_rep, op=A.mult)
            lfbig = sb.tile([P, NCH * K], f32, tag="lfbig")
            nc.scalar.activation(lfbig[:], fm1[:], ACT.Ln, bias=1.0)

            # rf = exp(-lf) = 1/f, one ScalarE op over all chunks
            rfbig = sb.tile([P, NCH * K], f32, tag="rfbig")
            nc.scalar.activation(rfbig[:], lfbig[:], ACT.Exp, scale=-1.0)

            # slog_k = sum_t ln f: cross-chunk sum on DVE (strided view),
            # then one 64-col PE matmul over partitions
            lfsum = sb.tile([P, K], f32, tag="lfsum")
            nc.vector.tensor_reduce(
                lfsum[:], lfbig.rearrange("p (c k) -> p k c", c=NCH),
                axis=mybir.AxisListType.X, op=A.add)
            slog_ps = ps.tile([1, K], f32, tag="slog")
            nc.tensor.matmul(slog_ps[:], onescol[:], lfsum[:],
                             start=True, stop=True)

            # G = exp(slog); gw = w * G; partition-broadcast on GpSimd
            g = sb.tile([1, K], f32, tag="g")
            nc.scalar.activation(g[:], slog_ps[:], ACT.Exp)
            gw = sb.tile([1, K], f32, tag="gw")
            nc.vector.tensor_tensor(gw[:], g[:], rows[:, 0:K], op=A.mult)
            gwbc = sb.tile([P, K], f32, tag="gwbc")
            nc.gpsimd.partition_broadcast(gwbc[:], gw[:], channels=P)

            # cfin[t,c] = p[c,t] * sum_k rf * gwbc  (q product + k-reduce)
            gwbc_rep = gwbc.unsqueeze(1).broadcast_to([P, NCH, K])
            q = sb.tile([P, NCH * K], f32, tag="q")
            nc.vector.tensor_tensor(q.rearrange("p (c k) -> p c k", c=NCH),
                                    rfbig.rearrange("p (c k) -> p c k", c=NCH),
                                    gwbc_rep, op=A.mult)
            cfin8 = sb.tile([P, NCH], f32, tag="cfin8")
            nc.vector.tensor_reduce(
                cfin8[:], q.rearrange("p (c k) -> p c k", c=NCH),
                axis=mybir.AxisListType.X, op=A.add)
            cfin = sb.tile([P, NCH], f32, tag="cfin")
            nc.vector.tensor_tensor(cfin[:], cfin8[:], pcol[:], op=A.mult)

            # z = sum_c cfin_c * x_c  (DVE chain), out = ones^T @ z
            zpp = [sb.tile([P, D], f32, tag="z0", name="z0"),
                   sb.tile([P, D], f32, tag="z1", name="z1")]
            for c in range(NCH):
                xc = xall[:, c * D:(c + 1) * D]
                if c == 0:
                    nc.vector.tensor_scalar_mul(zpp[0][:], xc,
                                                cfin[:, 0:1])
                else:
                    nc.vector.scalar_tensor_tensor(zpp[c % 2][:], xc,
                                                   cfin[:, c:c + 1],
                                                   zpp[(c - 1) % 2][:],
                                                   op0=A.mult, op1=A.add)

            out_ps = ps.tile([1, D], f32, tag="out")
            nc.tensor.matmul(out_ps[:], onescol[:], zpp[(NCH - 1) % 2][:],
                             start=True, stop=True)
            out_sb = sb.tile([1, D], f32, tag="outsb")
            nc.vector.tensor_copy(out_sb[:], out_ps[:])
            nc.sync.dma_start(out_d[:], out_sb[:])

    nc.compile()
    return nc


def _make_in_map(p, x):
    p = np.ascontiguousarray(np.asarray(p, dtype=np.float32)).reshape(T)
    x = np.ascontiguousarray(np.asarray(x, dtype=np.float32)).reshape(T, D)
    u, w = _gl_nodes_weights()
    um1bc = np.tile((u - 1.0).astype(np.float32)[None, :], (P, 1))
    pcol = np.ascontiguousarray(p.reshape(NCH, P).T)
    onescol = np.ones((P, 1), np.float32)
    aux = np.ascontiguousarray(
        np.concatenate([um1bc, pcol, onescol], axis=1))
    rows = np.concatenate([w.astype(np.float32),
                           np.ones(P, np.float32)]).reshape(1, K + P)
    xall = np.ascontiguousarray(
        x.reshape(NCH, P, D).transpose(1, 0, 2).reshape(P, NCH * D))
    return {
        "xall": xall,
        "aux": aux,
        "rows": rows,
    }


def _run(p, x, trace=False, tmpdir=None):
    from concourse.bass_utils import run_bass_kernel_spmd

    if "nc" not in _CACHE:
        _CACHE["nc"] = _build_program()
    nc = _CACHE["nc"]
    in_map = _make_in_map(p, x)
    in_maps = [in_map for _ in range(N_CORES)]
    res = run_bass_kernel_spmd(nc, in_maps, list(range(N_CORES)),
                               trace=trace, tmpdir=tmpdir)
    out = np.asarray(res.results[0]["out"], dtype=np.float32).reshape(D)
    return out, res


def kernel(p, x):
    out, _ = _run(p, x, trace=False)
    return out
